# revision 1
# baseline (speedup 1.0000x reference)
"""Trainium2 Bass kernel for nn_BinaryTokenClassificationModel (segment_reduce).

Math: the reference pools token embeddings into word embeddings (mean over
contiguous runs of equal word ids), then computes
    logits[b,s,t] = src_pooled[b,s] @ w_src + tgt_pooled[b,t] @ w_tgt + b.
Because the classifier is linear, pooling and projection commute:
    src_proj[w] = sum_t A[w,t] * (tok_h[t] @ w_src)     (A = 1/count-weighted
    tgt_proj[w] = sum_t A[w,t] * (tok_h[t] @ w_tgt)      segment membership)
and the output is the outer sum src_proj[s] + tgt_proj[t] + b. Each core:
  1. streams its batch row of tok_h [512, 768] through a fused DVE
     multiply-reduce against the broadcast weight row -> u[t] (per-token scalar)
  2. builds the membership matrix on-device (GpSimd iota + compare against the
     per-token segment index) -- no membership DMA traffic
  3. accumulates  atw_c.T @ broadcast(u_c)  (src chunks) and
     broadcast(u_c).T @ atw_c  (tgt chunks) straight into the [S, T] output
     PSUM tile, which realizes segment-reduce + outer-sum in one matmul/chunk.
Data-parallel over batch: core i handles batch row i. No collectives.
"""

import functools

import numpy as np

import concourse.bacc as bacc
import concourse.mybir as mybir
from concourse.bass_utils import run_bass_kernel_spmd
from concourse.tile import TileContext
from concourse.tile_rust import add_dep_helper

# Problem geometry (hardcoded per spec)
B = 8
L_SRC = 256
L_TGT = 256
L = L_SRC + L_TGT  # 512
H = 768
P = 128            # SBUF partitions / tokens per chunk
NCHUNK = L // P    # 4
N_SRC_CHUNKS = L_SRC // P  # 2
N_CORES = 8
F32 = mybir.dt.float32


# ---------------------------------------------------------------------------
# Host-side segment bookkeeping (exact mirror of reference._pool_words)
# ---------------------------------------------------------------------------

def _segments(combined_wid, attention_mask, n_words):
    """Per-token dense run ids exactly as the reference computes them."""
    valid = (attention_mask > 0) & (combined_wid >= 0)  # [B, L]
    prev_wid = np.concatenate(
        [np.full((combined_wid.shape[0], 1), -2, dtype=combined_wid.dtype),
         combined_wid[:, :-1]], axis=1)
    prev_valid = np.concatenate(
        [np.zeros((valid.shape[0], 1), dtype=bool), valid[:, :-1]], axis=1)
    new_run = valid & ((combined_wid != prev_wid) | (~prev_valid))
    run_id = np.cumsum(new_run.astype(np.int64), axis=1) - 1  # [B, L]
    seg = np.where(valid, run_id, n_words)  # n_words = dummy slot
    return seg, valid


def _seg_weights(seg, valid, n_words):
    """1/max(count,1) weight for each token's segment (0 for invalid)."""
    Bv, Lv = seg.shape
    wgt = np.zeros((Bv, Lv), dtype=np.float32)
    for b in range(Bv):
        counts = np.bincount(seg[b][valid[b]], minlength=Lv + 1).astype(np.float32)
        inv = 1.0 / np.maximum(counts, 1.0)
        wgt[b] = np.where(valid[b] & (seg[b] < n_words), inv[np.minimum(seg[b], Lv)], 0.0)
    return wgt


def _host_forward(tok_h, attention_mask, source_word_ids, target_word_ids, W, b, S, T):
    """Pure numpy forward implementing the same algebra the device runs.

    Used for validation only (test harness); not called by kernel().
    """
    combined = np.concatenate([source_word_ids, target_word_ids], axis=1).astype(np.int64)
    seg, valid = _segments(combined, np.asarray(attention_mask), S + T)
    wgt = _seg_weights(seg, valid, S + T)
    w_src = W[:H, 0].astype(np.float32)
    w_tgt = W[H:2 * H, 0].astype(np.float32)
    out = np.empty((tok_h.shape[0], S, T), dtype=np.float32)
    for bi in range(tok_h.shape[0]):
        u_src = tok_h[bi].astype(np.float32) @ w_src  # [L]
        u_tgt = tok_h[bi].astype(np.float32) @ w_tgt  # [L]
        proj = np.zeros(S + T, dtype=np.float32)
        for t in range(L):
            s = seg[bi, t]
            if s < S:
                proj[s] += wgt[bi, t] * u_src[t]
            elif s < S + T:
                proj[s] += wgt[bi, t] * u_tgt[t]
        out[bi] = proj[:S, None] + proj[None, S:S + T] + float(np.asarray(b).reshape(-1)[0])
    return out


# ---------------------------------------------------------------------------
# Device kernel, fast path (block_ok): src tokens -> word rows [0,S),
# tgt tokens -> word rows [S,S+T)
# ---------------------------------------------------------------------------

def _declare_block_params(nc, S, T):
    MW = 2 * NCHUNK  # meta columns
    return dict(
        tok0=nc.declare_dram_parameter("tok0", [P, H + MW], F32, isOutput=False),
        tok1=nc.declare_dram_parameter("tok1", [P, H], F32, isOutput=False),
        tok2=nc.declare_dram_parameter("tok2", [P, H], F32, isOutput=False),
        tok3=nc.declare_dram_parameter("tok3", [P, H], F32, isOutput=False),
        # wcat = [w_src (H) | w_tgt (H) | bias (1)]
        wcat=nc.declare_dram_parameter("wcat", [1, 2 * H + 1], F32, isOutput=False),
        # iotac[p, w] = w  (constant; GpSimd iota is a slow SW op on HW)
        iotac=nc.declare_dram_parameter("iotac", [P, P], F32, isOutput=False),
        out=nc.declare_dram_parameter("out", [S, T], F32, isOutput=True),
    )


def _emit_block_body(nc, tc, prm, S, T, mm_mode="mat", prod_space="SBUF"):
    """Fast path. DMA layout: chunk 0 and chunk 3 token loads are split in
    half-rows -- chunk 0 so the (tiny, pipeline-gating) wcat transfer can slip
    into the DMA stream between the halves, chunk 3 so the tail reduce after
    the last byte lands is half-length. meta rides as extra columns packed
    into the first token piece (no DMA of its own)."""
    MW = 2 * NCHUNK
    tok0, tok1, tok2, tok3, wcat, iotac, out = (
        prm["tok0"], prm["tok1"], prm["tok2"], prm["tok3"],
        prm["wcat"], prm["iotac"], prm["out"])
    if True:
        with (
            tc.tile_pool(name="const", bufs=1) as cpool,
            tc.tile_pool(name="toks", bufs=6) as tpool,
            tc.tile_pool(name="prods", bufs=2) as ppool,
            tc.tile_pool(name="atws", bufs=2) as apool,
            tc.tile_pool(name="psum", bufs=1, space="PSUM") as pspool,
        ):
            # wcat rides the SWDGE (Pool) path so it never contends with the
            # token loads for HWDGE issue slots; it gates the weight
            # broadcasts which gate the whole DVE pipeline.
            with tc.high_priority():
                wcat_sb = cpool.tile([1, 2 * H + 1], F32)
                nc.scalar.dma_start(out=wcat_sb[:], in_=wcat[:])

            # token chunk loads own the SP HWDGE queue (~75% of all bytes)
            t0 = tpool.tile([P, H + MW], F32)
            nc.sync.dma_start(out=t0[:], in_=tok0[:])
            t1 = tpool.tile([P, H], F32)
            nc.sync.dma_start(out=t1[:], in_=tok1[:])
            t2 = tpool.tile([P, H], F32)
            nc.sync.dma_start(out=t2[:], in_=tok2[:])
            t3 = tpool.tile([P, H], F32)
            nc.sync.dma_start(out=t3[:], in_=tok3[:])
            meta_sb = t0[:, H:H + MW]

            # weight rows broadcast across partitions (GpSimd cross-partition
            # op; overlaps the token DMA stream)
            wb_src = cpool.tile([P, H], F32)
            wb_tgt = cpool.tile([P, H], F32)
            with tc.high_priority():
                nc.gpsimd.partition_broadcast(wb_src[:], wcat_sb[0:1, 0:H])
                nc.gpsimd.partition_broadcast(wb_tgt[:], wcat_sb[0:1, H:2 * H])

            # iota constant rides the idle ACT DGE queue
            iota_f = cpool.tile([P, P], F32)
            nc.scalar.dma_start(out=iota_f[:], in_=iotac[:])

            # bias column [S, 1]: broadcast b down the partitions (Pool,
            # off the critical path); added during the final copy-out
            bias_col = cpool.tile([P, 1], F32)
            nc.gpsimd.partition_broadcast(bias_col[:], wcat_sb[0:1, 2 * H:2 * H + 1])

            if mm_mode == "mat":
                ones_pt = cpool.tile([P, P], F32)
                nc.vector.memset(ones_pt[:], 1.0)

            psum_out = pspool.tile([S, T], F32)

            # per-chunk token pieces: (tile, column range) pairs
            chunk_pieces = [
                [(t0, 0, H)],
                [(t1, 0, H)],
                [(t2, 0, H)],
                [(t3, 0, H)],
            ]

            # membership tiles: atw_c[t, w] = (seg[t] == w) * wgt[t].
            # Built on DVE (GpSimd tensor_scalar is ~2.4us/op on HW), before
            # the reduce chain so they are off the critical tail.
            atw_tiles = []
            for c in range(NCHUNK):
                width = S if c < N_SRC_CHUNKS else T
                atw_c = apool.tile([P, P], F32, name=f"atw_{c}")
                nc.vector.tensor_scalar(
                    out=atw_c[:, :width], in0=iota_f[:, :width],
                    scalar1=meta_sb[:, 2 * c:2 * c + 1],
                    scalar2=meta_sb[:, 2 * c + 1:2 * c + 2],
                    op0=mybir.AluOpType.is_equal, op1=mybir.AluOpType.mult)
                atw_tiles.append(atw_c)

            u_sb = cpool.tile([P, 2 * NCHUNK], F32)
            scratch_col = NCHUNK
            for c in range(NCHUNK):
                is_src = c < N_SRC_CHUNKS
                width = S if is_src else T
                wb = wb_src if is_src else wb_tgt

                # u_c[t] = tok_c[t, :] . w  -- fused multiply+reduce on DVE
                # (AFFINE_MUL_REDUCE custom op; seed is 0 so multi-piece
                # chunks sum their partials with one [P,1] add)
                pieces = chunk_pieces[c]
                accs = []
                for pi, (tile_, j0, j1) in enumerate(pieces):
                    if len(pieces) == 1:
                        acc = u_sb[:, c:c + 1]
                    else:
                        acc = u_sb[:, scratch_col:scratch_col + 1]
                        scratch_col += 1
                    prod = ppool.tile([P, j1 - j0], F32, name=f"prod_{c}_{pi}",
                                      space=prod_space)
                    nc.vector.affine_mul_reduce(
                        out=prod[:], accum_out=acc, in0=tile_[:, 0:j1 - j0],
                        in1=wb[:, j0:j1], scale=1.0, bias=0.0)
                    accs.append(acc)
                if len(accs) > 1:
                    nc.vector.tensor_tensor(
                        out=u_sb[:, c:c + 1], in0=accs[0], in1=accs[1],
                        op=mybir.AluOpType.add)

                atw_c = atw_tiles[c]
                u_b = u_sb[:, c:c + 1]
                if mm_mode == "mat":
                    ub_mat = ppool.tile([P, P], F32, name=f"ubm_{c}", tag="ubm")
                    nc.vector.tensor_scalar_mul(ub_mat[:], ones_pt[:], u_b)
                    rhs_b, lhs_b = ub_mat[:, :T], ub_mat[:, :S]
                else:
                    rhs_b, lhs_b = u_b.broadcast_to([P, T]), u_b.broadcast_to([P, S])
                first = c == 0
                last = c == NCHUNK - 1
                if is_src:
                    # psum[s, t] += sum_t' atw[t', s] * u[t']  (same for all t)
                    nc.tensor.matmul(
                        psum_out[:], atw_c[:, :S], rhs_b,
                        start=first, stop=last)
                else:
                    nc.tensor.matmul(
                        psum_out[:], lhs_b, atw_c[:, :T],
                        start=first, stop=last)

            out_sb = cpool.tile([S, T], F32)
            nc.vector.tensor_scalar_add(out_sb[:], psum_out[:], bias_col[0:S, :])
            nc.sync.dma_start(out=out[:], in_=out_sb[:])


# ---------------------------------------------------------------------------
# Device kernel, general fallback: tokens may map into either word block
# ---------------------------------------------------------------------------

def _build_general(nc, S, T):
    NW = S + T
    tok = nc.declare_dram_parameter("tok", [L, H], F32, isOutput=False)
    atw = nc.declare_dram_parameter("atw", [NCHUNK, P, NW], F32, isOutput=False)
    wcat = nc.declare_dram_parameter("wcat", [1, 2 * H + 1], F32, isOutput=False)
    out = nc.declare_dram_parameter("out", [S, T], F32, isOutput=True)

    with TileContext(nc) as tc:
        with (
            tc.tile_pool(name="const", bufs=1) as cpool,
            tc.tile_pool(name="toks", bufs=3) as tpool,
            tc.tile_pool(name="prods", bufs=2) as ppool,
            tc.tile_pool(name="atws", bufs=2) as apool,
            tc.tile_pool(name="psum", bufs=1, space="PSUM") as pspool,
        ):
            wcat_sb = cpool.tile([1, 2 * H + 1], F32)
            nc.gpsimd.dma_start(out=wcat_sb[:], in_=wcat[:])
            ones = cpool.tile([1, P], F32)
            nc.vector.memset(ones[:], 1.0)
            bias_row = cpool.tile([1, T], F32)
            nc.vector.tensor_scalar_mul(
                bias_row[:], ones[:, :T], wcat_sb[0:1, 2 * H:2 * H + 1])

            wb_src = pspool.tile([P, H], F32)
            wb_tgt = pspool.tile([P, H], F32)
            for wb, w0 in ((wb_src, 0), (wb_tgt, H)):
                for j0, j1 in ((0, 512), (512, H)):
                    nc.tensor.matmul(
                        wb[:, j0:j1], ones[:, :P], wcat_sb[0:1, w0 + j0:w0 + j1],
                        start=True, stop=True)

            psum_out = pspool.tile([S, T], F32)
            nc.tensor.matmul(psum_out[:], ones[:, :S], bias_row[:],
                             start=True, stop=False)

            u_src_sb = cpool.tile([P, NCHUNK], F32)
            u_tgt_sb = cpool.tile([P, NCHUNK], F32)
            for c in range(NCHUNK):
                tok_c = tpool.tile([P, H], F32, name=f"tok_{c}")
                nc.sync.dma_start(out=tok_c[:], in_=tok[c * P:(c + 1) * P, :])
                for kind, wb, usb in (("s", wb_src, u_src_sb), ("t", wb_tgt, u_tgt_sb)):
                    prod = ppool.tile([P, H], F32, name=f"prod_{kind}_{c}")
                    nc.vector.affine_mul_reduce(
                        out=prod[:], accum_out=usb[:, c:c + 1], in0=tok_c[:],
                        in1=wb[:], scale=1.0, bias=0.0)

                atw_c = apool.tile([P, NW], F32, name=f"atw_{c}")
                nc.sync.dma_start(out=atw_c[:], in_=atw[c])
                last = c == NCHUNK - 1
                nc.tensor.matmul(
                    psum_out[:], atw_c[:, :S], u_src_sb[:, c:c + 1].broadcast_to([P, T]),
                    start=False, stop=False)
                nc.tensor.matmul(
                    psum_out[:], u_tgt_sb[:, c:c + 1].broadcast_to([P, S]), atw_c[:, S:],
                    start=False, stop=last)

            out_sb = cpool.tile([S, T], F32)
            nc.vector.tensor_scalar_add(out_sb[:], psum_out[:], bias_col[0:S, :])
            nc.sync.dma_start(out=out[:], in_=out_sb[:])


# variant knobs (fixed at import for the graded path; bench overrides)
MM_MODE = "mat"
PROD_SPACE = "SBUF"


@functools.lru_cache(maxsize=4)
def _build(S, T, block_ok, mm_mode=None, prod_space=None):
    mm_mode = MM_MODE if mm_mode is None else mm_mode
    prod_space = PROD_SPACE if prod_space is None else prod_space
    nc = bacc.Bacc("TRN2", debug=False, num_devices=N_CORES)
    if block_ok:
        prm = _declare_block_params(nc, S, T)
        with TileContext(nc) as tc:
            _emit_block_body(nc, tc, prm, S, T, mm_mode, prod_space)
    else:
        _build_general(nc, S, T)
    nc.compile()
    return nc


@functools.lru_cache(maxsize=16)
def _build_looped(S, T, iters, mm_mode=None, prod_space=None):
    """Timing-only variant: the same body repeated `iters` times inside one
    NEFF via a Tile For_i loop (per-iteration all-engine barrier back-edge)."""
    mm_mode = MM_MODE if mm_mode is None else mm_mode
    prod_space = PROD_SPACE if prod_space is None else prod_space
    nc = bacc.Bacc("TRN2", debug=False, num_devices=N_CORES)
    prm = _declare_block_params(nc, S, T)
    with TileContext(nc) as tc:
        with tc.For_i(0, iters, 1):
            _emit_block_body(nc, tc, prm, S, T, mm_mode, prod_space)
    nc.compile()
    return nc


# ---------------------------------------------------------------------------
# Host wrapper
# ---------------------------------------------------------------------------

def _prep(inputs):
    tok_h = np.ascontiguousarray(np.asarray(inputs["tok_h"], dtype=np.float32))
    mask = np.asarray(inputs["attention_mask"])
    swid = np.asarray(inputs["source_word_ids"])
    twid = np.asarray(inputs["target_word_ids"])
    W = np.asarray(inputs["W"], dtype=np.float32)
    b = np.asarray(inputs["b"], dtype=np.float32)
    S = int(np.asarray(inputs["S"]))
    T = int(np.asarray(inputs["T"]))

    Bv, Lv, Hv = tok_h.shape
    assert (Bv, Lv, Hv) == (B, L, H), f"unexpected tok_h shape {tok_h.shape}"
    assert swid.shape == (B, L_SRC) and twid.shape == (B, L_TGT)
    assert S <= P and T <= P

    NW = S + T
    combined = np.concatenate([swid, twid], axis=1).astype(np.int64)
    seg, valid = _segments(combined, mask, NW)
    wgt = _seg_weights(seg, valid, NW)

    src_tok_seg = seg[:, :L_SRC][valid[:, :L_SRC]]
    tgt_tok_seg = seg[:, L_SRC:][valid[:, L_SRC:]]
    block_ok = bool(
        (src_tok_seg < S).all()
        and (tgt_tok_seg >= S).all() and (tgt_tok_seg < NW).all()
    )

    wcat = np.zeros((1, 2 * H + 1), dtype=np.float32)
    wcat[0, :H] = W[:H, 0]
    wcat[0, H:2 * H] = W[H:2 * H, 0]
    wcat[0, 2 * H] = b.reshape(-1)[0]

    in_maps = []
    if block_ok:
        # meta[b, t_local, 2c] = in-block segment col (or -1), [.., 2c+1] = wgt
        meta = np.zeros((B, P, 2 * NCHUNK), dtype=np.float32)
        for bi in range(B):
            for c in range(NCHUNK):
                tsl = slice(c * P, (c + 1) * P)
                segc = seg[bi, tsl].astype(np.int64)
                col = segc if c < N_SRC_CHUNKS else segc - S
                ok = valid[bi, tsl] & (segc < NW)
                meta[bi, :, 2 * c] = np.where(ok, col, -1).astype(np.float32)
                meta[bi, :, 2 * c + 1] = wgt[bi, tsl]
        for i in range(N_CORES):
            bi = i % B
            tk = tok_h[bi]
            in_maps.append({
                # chunk 0 carries meta as extra columns
                "tok0": np.ascontiguousarray(
                    np.concatenate([tk[0:P, :], meta[bi]], axis=1)),
                "tok1": np.ascontiguousarray(tk[P:2 * P, :]),
                "tok2": np.ascontiguousarray(tk[2 * P:3 * P, :]),
                "tok3": np.ascontiguousarray(tk[3 * P:4 * P, :]),
                "wcat": wcat,
                "iotac": np.tile(np.arange(P, dtype=np.float32), (P, 1)),
            })
    else:
        atw = np.zeros((B, NCHUNK, P, NW), dtype=np.float32)
        for bi in range(B):
            for t in range(L):
                s = seg[bi, t]
                if s >= NW or not valid[bi, t]:
                    continue
                atw[bi, t // P, t % P, s] = wgt[bi, t]
        for i in range(N_CORES):
            bi = i % B
            in_maps.append({"tok": tok_h[bi], "atw": atw[bi], "wcat": wcat})
    return S, T, block_ok, in_maps


def kernel(**inputs):
    S, T, block_ok, in_maps = _prep(inputs)
    nc = _build(S, T, block_ok)
    res = run_bass_kernel_spmd(nc, in_maps, core_ids=list(range(N_CORES)))
    return np.stack([res.results[i]["out"] for i in range(B)], axis=0)


@functools.lru_cache(maxsize=4)
def _build_looped_empty(iters):
    """Calibration: same For_i loop with a minimal body, to measure the
    per-iteration loop overhead (back-edge barrier + sem reset)."""
    nc = bacc.Bacc("TRN2", debug=False, num_devices=N_CORES)
    x = nc.declare_dram_parameter("x", [P, 16], F32, isOutput=False)
    y = nc.declare_dram_parameter("y", [P, 16], F32, isOutput=True)
    with TileContext(nc) as tc:
        with tc.tile_pool(name="p", bufs=2) as pool:
            t = pool.tile([P, 16], F32)
            nc.sync.dma_start(out=t[:], in_=x[:])
            with tc.For_i(0, iters, 1):
                w = pool.tile([P, 16], F32)
                nc.vector.tensor_copy(w[:], t[:])
            nc.sync.dma_start(out=y[:], in_=t[:])
    nc.compile()
    return nc



# revision 2
# speedup vs baseline: 1.4257x; 1.4257x over previous
"""Trainium2 Bass kernel for nn_BinaryTokenClassificationModel (segment_reduce).

Math: the reference pools token embeddings into word embeddings (mean over
contiguous runs of equal word ids), then computes
    logits[b,s,t] = src_pooled[b,s] @ w_src + tgt_pooled[b,t] @ w_tgt + b.
Because the classifier is linear, pooling and projection commute:
    src_proj[w] = sum_t A[w,t] * (tok_h[t] @ w_src)     (A = 1/count-weighted
    tgt_proj[w] = sum_t A[w,t] * (tok_h[t] @ w_tgt)      segment membership)
and the output is the outer sum src_proj[s] + tgt_proj[t] + b.

Device pipeline (data-parallel, core i = batch row i, no collectives):
  - tok_h is shipped in bf16 (rel-err budget 2e-2; bf16 lands ~5e-3), halving
    the dominant HBM traffic. Four [128, 768] chunks stream on the SP HWDGE
    queue; the weight broadcast (wb), membership matrix (atw) and bias ride
    the Act HWDGE queue in parallel.
  - wb / atw are precomputed host-side: wb[p, :] = [w_src | w_tgt] for every
    partition p (replication only), atw[tok, word] = 1/count membership.
    This removes the on-device GpSimd broadcast + iota/compare chain that
    previously gated the whole pipeline.
  - per chunk: u_c[t] = tok_c[t, :] . w  via fused DVE multiply-reduce
    (f32 accumulate), then one bf16 TensorE matmul accumulates
    atw_c^T @ broadcast(u_c) (src) or broadcast(u_c) @ atw_c (tgt)
    into the [S, T] f32 PSUM tile = segment-reduce + outer-sum fused.
  - a dummy DVE custom op at program start pre-loads the DVE uop table so the
    first real mul-reduce doesn't pay the ~1us lazy table load.
  - output is DMA'd back in bf16 and upcast on host.
"""

import functools

import numpy as np
import ml_dtypes

import concourse.bacc as bacc
import concourse.mybir as mybir
from concourse.bass_utils import run_bass_kernel_spmd
from concourse.tile import TileContext

# Problem geometry (hardcoded per spec)
B = 8
L_SRC = 256
L_TGT = 256
L = L_SRC + L_TGT  # 512
H = 768
P = 128            # SBUF partitions / tokens per chunk
NCHUNK = L // P    # 4
N_SRC_CHUNKS = L_SRC // P  # 2
N_CORES = 8
F32 = mybir.dt.float32
BF16 = mybir.dt.bfloat16
NPBF16 = ml_dtypes.bfloat16


# ---------------------------------------------------------------------------
# Host-side segment bookkeeping (exact mirror of reference._pool_words)
# ---------------------------------------------------------------------------

def _segments(combined_wid, attention_mask, n_words):
    """Per-token dense run ids exactly as the reference computes them."""
    valid = (attention_mask > 0) & (combined_wid >= 0)  # [B, L]
    prev_wid = np.concatenate(
        [np.full((combined_wid.shape[0], 1), -2, dtype=combined_wid.dtype),
         combined_wid[:, :-1]], axis=1)
    prev_valid = np.concatenate(
        [np.zeros((valid.shape[0], 1), dtype=bool), valid[:, :-1]], axis=1)
    new_run = valid & ((combined_wid != prev_wid) | (~prev_valid))
    run_id = np.cumsum(new_run.astype(np.int64), axis=1) - 1  # [B, L]
    seg = np.where(valid, run_id, n_words)  # n_words = dummy slot
    return seg, valid


def _seg_weights(seg, valid, n_words):
    """1/max(count,1) weight for each token's segment (0 for invalid)."""
    Bv, Lv = seg.shape
    wgt = np.zeros((Bv, Lv), dtype=np.float32)
    for b in range(Bv):
        counts = np.bincount(seg[b][valid[b]], minlength=Lv + 1).astype(np.float32)
        inv = 1.0 / np.maximum(counts, 1.0)
        wgt[b] = np.where(valid[b] & (seg[b] < n_words), inv[np.minimum(seg[b], Lv)], 0.0)
    return wgt


# ---------------------------------------------------------------------------
# Device kernel, fast path (block_ok): src tokens -> word rows [0,S),
# tgt tokens -> word rows [S,S+T)
# ---------------------------------------------------------------------------

def _declare_block_params(nc, S, T):
    return dict(
        tok0=nc.declare_dram_parameter("tok0", [P, H], BF16, isOutput=False),
        tok1=nc.declare_dram_parameter("tok1", [P, H], BF16, isOutput=False),
        tok2=nc.declare_dram_parameter("tok2", [P, H], BF16, isOutput=False),
        tok3=nc.declare_dram_parameter("tok3", [P, H], BF16, isOutput=False),
        # wb[p, :H] = w_src, wb[p, H:] = w_tgt  (host-side broadcast)
        wb=nc.declare_dram_parameter("wb", [P, 2 * H], BF16, isOutput=False),
        # atwb[p, c*P + w] = (seg[c*P+p] == block word w) * wgt[c*P+p]
        atwb=nc.declare_dram_parameter("atwb", [P, NCHUNK * P], BF16, isOutput=False),
        bias=nc.declare_dram_parameter("bias", [P, 1], F32, isOutput=False),
        out=nc.declare_dram_parameter("out", [S, T], BF16, isOutput=True),
    )


def _emit_block_body(nc, tc, prm, S, T, mm_mode="mat"):
    tok_drams = [prm["tok0"], prm["tok1"], prm["tok2"], prm["tok3"]]
    wb, atwb, bias, out = prm["wb"], prm["atwb"], prm["bias"], prm["out"]
    with (
        tc.tile_pool(name="const", bufs=1) as cpool,
        tc.tile_pool(name="toks", bufs=4) as tpool,
        tc.tile_pool(name="prods", bufs=2) as ppool,
        tc.tile_pool(name="psum", bufs=1, space="PSUM") as pspool,
    ):
        # --- DVE uop-table warmup: tiny dummy custom op on a memset tile so
        # the first real affine_mul_reduce doesn't pay the lazy table load.
        warm = cpool.tile([1, 64], BF16)
        warm_acc = cpool.tile([1, 1], F32)
        nc.vector.memset(warm[:], 0.0)
        nc.vector.affine_mul_reduce(
            out=warm[:], accum_out=warm_acc[:], in0=warm[:], in1=warm[:],
            scale=1.0, bias=0.0)

        # --- Act HWDGE queue: weight broadcast first (gates the mul-reduce
        # chain), then membership + bias.
        with tc.high_priority():
            wb_sb = cpool.tile([P, 2 * H], BF16)
            nc.scalar.dma_start(out=wb_sb[:], in_=wb[:])
            atwb_sb = cpool.tile([P, NCHUNK * P], BF16)
            nc.scalar.dma_start(out=atwb_sb[:], in_=atwb[:])
            bias_sb = cpool.tile([P, 1], F32)
            nc.scalar.dma_start(out=bias_sb[:], in_=bias[:])

        # --- SP HWDGE queue: the four token chunks (bulk of the bytes).
        tok_sb = []
        for c in range(NCHUNK):
            t = tpool.tile([P, H], BF16, name=f"tok_{c}")
            nc.sync.dma_start(out=t[:], in_=tok_drams[c][:])
            tok_sb.append(t)

        if mm_mode == "mat":
            ones_pt = cpool.tile([P, P], BF16)
            nc.vector.memset(ones_pt[:], 1.0)

        psum_out = pspool.tile([S, T], F32)
        u_sb = cpool.tile([P, NCHUNK], F32)
        ub_bf = cpool.tile([P, NCHUNK], BF16)

        for c in range(NCHUNK):
            is_src = c < N_SRC_CHUNKS
            # u_c[t] = tok_c[t, :] . w  -- fused multiply+reduce on DVE
            wb_col = wb_sb[:, 0:H] if is_src else wb_sb[:, H:2 * H]
            prod = ppool.tile([P, H], BF16, name=f"prod_{c}")
            nc.vector.affine_mul_reduce(
                out=prod[:], accum_out=u_sb[:, c:c + 1], in0=tok_sb[c][:],
                in1=wb_col, scale=1.0, bias=0.0)

            atw_c = atwb_sb[:, c * P:(c + 1) * P]
            first = c == 0
            last = c == NCHUNK - 1
            if mm_mode == "mat":
                ub_mat = ppool.tile([P, P], BF16, name=f"ubm_{c}", tag="ubm")
                nc.vector.tensor_scalar_mul(ub_mat[:], ones_pt[:], u_sb[:, c:c + 1])
                rhs_b, lhs_b = ub_mat[:, :T], ub_mat[:, :S]
            else:
                nc.vector.tensor_copy(ub_bf[:, c:c + 1], u_sb[:, c:c + 1])
                rhs_b = ub_bf[:, c:c + 1].broadcast_to([P, T])
                lhs_b = ub_bf[:, c:c + 1].broadcast_to([P, S])
            if is_src:
                # psum[s, t] += sum_p atw[p, s] * u[p]   (same for all t)
                nc.tensor.matmul(psum_out[:], atw_c[:, :S], rhs_b,
                                 start=first, stop=last)
            else:
                nc.tensor.matmul(psum_out[:], lhs_b, atw_c[:, :T],
                                 start=first, stop=last)

        out_sb = cpool.tile([S, T], BF16)
        nc.vector.tensor_scalar_add(out_sb[:], psum_out[:], bias_sb[0:S, :])
        nc.sync.dma_start(out=out[:], in_=out_sb[:])


# ---------------------------------------------------------------------------
# Device kernel, general fallback: tokens may map into either word block
# ---------------------------------------------------------------------------

def _build_general(nc, S, T):
    NW = S + T
    tok = nc.declare_dram_parameter("tok", [L, H], F32, isOutput=False)
    atw = nc.declare_dram_parameter("atw", [NCHUNK, P, NW], F32, isOutput=False)
    wcat = nc.declare_dram_parameter("wcat", [1, 2 * H + 1], F32, isOutput=False)
    out = nc.declare_dram_parameter("out", [S, T], F32, isOutput=True)

    with TileContext(nc) as tc:
        with (
            tc.tile_pool(name="const", bufs=1) as cpool,
            tc.tile_pool(name="toks", bufs=3) as tpool,
            tc.tile_pool(name="prods", bufs=2) as ppool,
            tc.tile_pool(name="atws", bufs=2) as apool,
            tc.tile_pool(name="psum", bufs=1, space="PSUM") as pspool,
        ):
            wcat_sb = cpool.tile([1, 2 * H + 1], F32)
            nc.scalar.dma_start(out=wcat_sb[:], in_=wcat[:])
            ones = cpool.tile([1, P], F32)
            nc.vector.memset(ones[:], 1.0)
            bias_row = cpool.tile([1, T], F32)
            nc.vector.tensor_scalar_mul(
                bias_row[:], ones[:, :T], wcat_sb[0:1, 2 * H:2 * H + 1])

            wb_src = pspool.tile([P, H], F32)
            wb_tgt = pspool.tile([P, H], F32)
            for wb, w0 in ((wb_src, 0), (wb_tgt, H)):
                for j0, j1 in ((0, 512), (512, H)):
                    nc.tensor.matmul(
                        wb[:, j0:j1], ones[:, :P], wcat_sb[0:1, w0 + j0:w0 + j1],
                        start=True, stop=True)

            psum_out = pspool.tile([S, T], F32)
            nc.tensor.matmul(psum_out[:], ones[:, :S], bias_row[:],
                             start=True, stop=False)

            u_src_sb = cpool.tile([P, NCHUNK], F32)
            u_tgt_sb = cpool.tile([P, NCHUNK], F32)
            for c in range(NCHUNK):
                tok_c = tpool.tile([P, H], F32, name=f"tok_{c}")
                nc.sync.dma_start(out=tok_c[:], in_=tok[c * P:(c + 1) * P, :])
                for kind, wb, usb in (("s", wb_src, u_src_sb), ("t", wb_tgt, u_tgt_sb)):
                    prod = ppool.tile([P, H], F32, name=f"prod_{kind}_{c}")
                    nc.vector.affine_mul_reduce(
                        out=prod[:], accum_out=usb[:, c:c + 1], in0=tok_c[:],
                        in1=wb[:], scale=1.0, bias=0.0)

                atw_c = apool.tile([P, NW], F32, name=f"atw_{c}")
                nc.sync.dma_start(out=atw_c[:], in_=atw[c])
                last = c == NCHUNK - 1
                nc.tensor.matmul(
                    psum_out[:], atw_c[:, :S], u_src_sb[:, c:c + 1].broadcast_to([P, T]),
                    start=False, stop=False)
                nc.tensor.matmul(
                    psum_out[:], u_tgt_sb[:, c:c + 1].broadcast_to([P, S]), atw_c[:, S:],
                    start=False, stop=last)

            out_sb = cpool.tile([S, T], F32)
            nc.vector.tensor_copy(out_sb[:], psum_out[:])
            nc.sync.dma_start(out=out[:], in_=out_sb[:])


# variant knobs (fixed at import for the graded path; bench overrides)
MM_MODE = "mat"


@functools.lru_cache(maxsize=4)
def _build(S, T, block_ok, mm_mode=None):
    mm_mode = MM_MODE if mm_mode is None else mm_mode
    nc = bacc.Bacc("TRN2", debug=False, num_devices=N_CORES)
    if block_ok:
        prm = _declare_block_params(nc, S, T)
        with TileContext(nc) as tc:
            _emit_block_body(nc, tc, prm, S, T, mm_mode)
    else:
        _build_general(nc, S, T)
    nc.compile()
    return nc


# ---------------------------------------------------------------------------
# Host wrapper
# ---------------------------------------------------------------------------

def _prep(inputs):
    tok_h = np.ascontiguousarray(np.asarray(inputs["tok_h"], dtype=np.float32))
    mask = np.asarray(inputs["attention_mask"])
    swid = np.asarray(inputs["source_word_ids"])
    twid = np.asarray(inputs["target_word_ids"])
    W = np.asarray(inputs["W"], dtype=np.float32)
    b = np.asarray(inputs["b"], dtype=np.float32)
    S = int(np.asarray(inputs["S"]))
    T = int(np.asarray(inputs["T"]))

    Bv, Lv, Hv = tok_h.shape
    assert (Bv, Lv, Hv) == (B, L, H), f"unexpected tok_h shape {tok_h.shape}"
    assert swid.shape == (B, L_SRC) and twid.shape == (B, L_TGT)
    assert S <= P and T <= P

    NW = S + T
    combined = np.concatenate([swid, twid], axis=1).astype(np.int64)
    seg, valid = _segments(combined, mask, NW)
    wgt = _seg_weights(seg, valid, NW)

    src_tok_seg = seg[:, :L_SRC][valid[:, :L_SRC]]
    tgt_tok_seg = seg[:, L_SRC:][valid[:, L_SRC:]]
    block_ok = bool(
        (src_tok_seg < S).all()
        and (tgt_tok_seg >= S).all() and (tgt_tok_seg < NW).all()
    )

    in_maps = []
    if block_ok:
        tok_bf = tok_h.astype(NPBF16)                       # [B, L, H]
        wb_row = np.concatenate([W[:H, 0], W[H:2 * H, 0]])  # [2H]
        wb_full = np.ascontiguousarray(
            np.broadcast_to(wb_row, (P, 2 * H))).astype(NPBF16)
        bias_col = np.full((P, 1), float(b.reshape(-1)[0]), dtype=np.float32)

        # atwb[b, p, c*P + col] = wgt for token c*P+p's in-block word col
        atwb = np.zeros((B, P, NCHUNK * P), dtype=np.float32)
        for bi in range(B):
            for c in range(NCHUNK):
                tsl = slice(c * P, (c + 1) * P)
                segc = seg[bi, tsl].astype(np.int64)
                col = segc if c < N_SRC_CHUNKS else segc - S
                ok = valid[bi, tsl] & (segc < NW) & (col >= 0) & (col < P)
                rows = np.arange(P)[ok]
                atwb[bi, rows, c * P + col[ok]] = wgt[bi, tsl][ok]
        atwb = atwb.astype(NPBF16)

        for i in range(N_CORES):
            bi = i % B
            tk = tok_bf[bi]
            in_maps.append({
                "tok0": np.ascontiguousarray(tk[0:P, :]),
                "tok1": np.ascontiguousarray(tk[P:2 * P, :]),
                "tok2": np.ascontiguousarray(tk[2 * P:3 * P, :]),
                "tok3": np.ascontiguousarray(tk[3 * P:4 * P, :]),
                "wb": wb_full,
                "atwb": atwb[bi],
                "bias": bias_col,
            })
    else:
        wcat = np.zeros((1, 2 * H + 1), dtype=np.float32)
        wcat[0, :H] = W[:H, 0]
        wcat[0, H:2 * H] = W[H:2 * H, 0]
        wcat[0, 2 * H] = b.reshape(-1)[0]
        atw = np.zeros((B, NCHUNK, P, NW), dtype=np.float32)
        for bi in range(B):
            for t in range(L):
                s = seg[bi, t]
                if s >= NW or not valid[bi, t]:
                    continue
                atw[bi, t // P, t % P, s] = wgt[bi, t]
        for i in range(N_CORES):
            bi = i % B
            in_maps.append({"tok": tok_h[bi], "atw": atw[bi], "wcat": wcat})
    return S, T, block_ok, in_maps


def kernel(**inputs):
    S, T, block_ok, in_maps = _prep(inputs)
    nc = _build(S, T, block_ok)
    res = run_bass_kernel_spmd(nc, in_maps, core_ids=list(range(N_CORES)))
    return np.stack(
        [np.asarray(res.results[i]["out"]).astype(np.float32) for i in range(B)],
        axis=0)
